# revision 22
# baseline (speedup 1.0000x reference)
"""Trainium2 Bass kernel for nn_Block_45518063403599 (dense transformer block).

Full inputs -> full outputs. Sharding: 8 cores = (batch b in 0..3) x (query
half in 0..1). Each core computes the block for its 1024 query tokens of its
batch (k/v over all 2048 tokens of that batch are recomputed per core pair --
zero cross-core communication, single SPMD launch).

Reference (eval mode):
    h  = LN1(x);  q,k,v = h @ Wq/Wk/Wv (per-head)
    attn = softmax(q k^T / 8);  o = attn @ v
    x1 = x + concat_heads(o) @ Wp + bp
    ff = relu(LN2(x1) @ W1 + b1) @ W2 + b2
    out = (x1 + ff, attn stacked [H,B,T,T])
"""
import os as _os
import numpy as np
import ml_dtypes

ABLATE = _os.environ.get("KABLATE", "")

import concourse.bass as bass
import concourse.tile as tile
import concourse.mybir as mybir
from concourse import bacc
from concourse.bass_utils import run_bass_kernel_spmd
from concourse.masks import make_identity

F32 = mybir.dt.float32
F32R = mybir.dt.float32r
BF16 = mybir.dt.bfloat16
AF = mybir.ActivationFunctionType
ALU = mybir.AluOpType

B, T, D, H = 4, 2048, 1024, 16
HD = D // H            # 64
TQ = T // 2            # 1024 query tokens per core
J = 4 * D              # 4096
P = 128
DT = D // P            # 8 d-tiles
TT = TQ // P           # 8 query-token tiles
ST = T // P            # 16 kv-token tiles
JT = J // P            # 32 ffn-hidden tiles
LN_EPS = 1e-5
SCALE = float(HD) ** -0.5   # 1/8


def _ln_transpose(nc, eps, xsrc, ntiles, dst, g, b, idf, pools):
    """LayerNorm rows of xsrc and write the transposed result into dst
    ([128, DT*ntiles*128] f32r), applying per-d affine (g, b) post-transpose.
    xsrc: either a DRAM AP [ntiles*128, D] or a fn(i)->SBUF AP [128, D]."""
    xpool, statpool, pstr = pools
    ncols = ntiles * P
    for i in range(ntiles):
        if callable(xsrc):
            xt = xsrc(i)
        else:
            xtile = xpool.tile([P, D], F32, tag="ln_x")
            nc.sync.dma_start(xtile[:], xsrc[i * P:(i + 1) * P, :])
            xt = xtile[:]
        s1 = statpool.tile([P, 1], F32, tag="ln_s1")
        nc.vector.tensor_reduce(s1[:], xt, axis=mybir.AxisListType.X, op=ALU.add)
        negmu = statpool.tile([P, 1], F32, tag="ln_negmu")
        nc.vector.tensor_scalar_mul(negmu[:], s1[:], -1.0 / D)
        sq = xpool.tile([P, D], F32, tag="ln_sq")
        ssq = statpool.tile([P, 1], F32, tag="ln_ssq")
        nc.scalar.activation(sq[:], xt, AF.Square, bias=negmu[:], scale=1.0,
                             accum_out=ssq[:])
        std = statpool.tile([P, 1], F32, tag="ln_std")
        nc.scalar.activation(std[:], ssq[:], AF.Sqrt, bias=eps[:], scale=1.0 / D)
        rstd = statpool.tile([P, 1], F32, tag="ln_rstd")
        nc.vector.reciprocal(rstd[:], std[:])
        cn = xpool.tile([P, D], F32, tag="ln_cn")
        nc.vector.tensor_scalar(cn[:], xt, negmu[:], rstd[:],
                                op0=ALU.add, op1=ALU.mult)
        for k in range(DT):
            pt = pstr.tile([P, P], F32, tag="ln_ps")
            nc.tensor.transpose(pt[:], cn[:, k * P:(k + 1) * P], idf[:])
            nc.vector.tensor_scalar(
                dst[:, k * ncols + i * P: k * ncols + (i + 1) * P], pt[:],
                g[:, k:k + 1], b[:, k:k + 1], op0=ALU.mult, op1=ALU.add)


def _proj_to_rows(nc, w_dram, src, src_cols, dst, dst_cols, psp, wpool):
    """dst[e, n] = sum_d w[d, e] * src[d, n] for e in 0..1023 (f32r matmuls).

    src: SBUF [128, DT*src_cols] f32r (col-block = d-tile).
    dst: SBUF [128, DT*dst_cols] (col-block = e-tile); dst_cols==src_cols.
    Streams w in 256-wide e-quarters."""
    nch = src_cols // 512
    for qe in range(4):
        wh = wpool.tile([P, DT * 256], F32R, tag="w")
        for dt_ in range(DT):
            nc.sync.dma_start(
                wh[:, dt_ * 256:(dt_ + 1) * 256],
                w_dram[dt_ * P:(dt_ + 1) * P,
                       qe * 256:(qe + 1) * 256].bitcast(F32R))
        for el in range(2):
            et = qe * 2 + el
            for ch in range(nch):
                ps = psp.tile([P, 512], F32, tag="ps")
                for dt_ in range(DT):
                    nc.tensor.matmul(
                        ps[:],
                        wh[:, dt_ * 256 + el * P: dt_ * 256 + (el + 1) * P],
                        src[:, dt_ * src_cols + ch * 512: dt_ * src_cols + (ch + 1) * 512],
                        start=(dt_ == 0), stop=(dt_ == DT - 1))
                nc.vector.tensor_copy(
                    dst[:, et * dst_cols + ch * 512: et * dst_cols + (ch + 1) * 512],
                    ps[:])


def build_nc():
    nc = bacc.Bacc("TRN2", target_bir_lowering=False, debug=False)

    t = {}
    t["xf"] = nc.dram_tensor("xf", [T, D], F32, kind="ExternalInput").ap()
    t["xq"] = nc.dram_tensor("xq", [TQ, D], F32, kind="ExternalInput").ap()
    for n in ["ln1g", "ln1b", "ln2g", "ln2b", "bp", "b2"]:
        t[n] = nc.dram_tensor(n, [D], F32, kind="ExternalInput").ap()
    t["b1"] = nc.dram_tensor("b1", [J], F32, kind="ExternalInput").ap()
    for n in ["wq", "wk", "wv"]:
        t[n] = nc.dram_tensor(n, [D, D], F32, kind="ExternalInput").ap()
    t["wp"] = nc.dram_tensor("wp", [D, D], BF16, kind="ExternalInput").ap()
    t["w1"] = nc.dram_tensor("w1", [D, J], F32, kind="ExternalInput").ap()
    t["w2"] = nc.dram_tensor("w2", [J, D], BF16, kind="ExternalInput").ap()
    t["y"] = nc.dram_tensor("y", [TQ, D], F32, kind="ExternalOutput").ap()
    t["attn"] = nc.dram_tensor("attn", [H, TQ, T], F32, kind="ExternalOutput").ap()

    with tile.TileContext(nc) as tc:
        _emit(nc, tc, t)
    nc.compile()
    return nc


def _emit(nc, tc, t):
    xf, xq, y, attn = t["xf"], t["xq"], t["y"], t["attn"]

    cm_const = tc.tile_pool(name="const", bufs=1)
    const = cm_const.__enter__()
    idf = const.tile([P, P], F32)
    make_identity(nc, idf[:])
    id16 = const.tile([P, P], BF16)
    make_identity(nc, id16[:])
    epsc = const.tile([P, 1], F32)
    nc.gpsimd.memset(epsc[:], LN_EPS)
    small = {}
    for name, n in [("ln1g", D), ("ln1b", D), ("ln2g", D), ("ln2b", D),
                    ("b1", J)]:
        s = const.tile([P, n // P], F32, tag=name)
        nc.sync.dma_start(s[:], t[name].rearrange("(k p) -> p k", p=P))
        small[name] = s
    bcast = {}
    for name in ["bp", "b2"]:
        row = const.tile([1, D], F32, tag=f"{name}_row")
        nc.sync.dma_start(row[:], t[name][None, :])
        full = const.tile([P, D], F32, tag=f"{name}_b")
        nc.gpsimd.partition_broadcast(full[:], row[:])
        bcast[name] = full

    # ---------- Phase 1: LN1(xf) -> hT [d, s] f32r (64KB/part) ------------
    cm_kv = tc.tile_pool(name="kv", bufs=1)
    kv_pool = cm_kv.__enter__()
    kT = kv_pool.tile([P, DT * T], F32R)        # [e, s] 64KB/part
    v16 = kv_pool.tile([P, ST * D], BF16)       # [s, e] 32KB/part

    cm_hT = tc.tile_pool(name="hT", bufs=1)
    hT_pool = cm_hT.__enter__()
    hT = hT_pool.tile([P, DT * T], F32R)
    with tc.tile_pool(name="ln1", bufs=2) as xpool, \
         tc.tile_pool(name="ln1s", bufs=4) as statpool, \
         tc.tile_pool(name="ln1p", bufs=4, space="PSUM") as pstr:
        _ln_transpose(nc, epsc, xf, ST, hT[:], small["ln1g"][:],
                      small["ln1b"][:], idf, (xpool, statpool, pstr))

    # ---------- Phase 2: k^T = Wk^T h^T -----------------------------------
    with tc.tile_pool(name="wkh", bufs=2) as wpool, \
         tc.tile_pool(name="qkvp", bufs=4, space="PSUM") as psqkv:
        _proj_to_rows(nc, t["wk"], hT[:], T, kT[:], T, psqkv, wpool)
        # ---------- Phase 3: v = h Wv (layout [s, e]) ---------------------
        for qe in range(4):
            wh = wpool.tile([P, DT * 256], F32R, tag="w")
            for dt_ in range(DT):
                nc.sync.dma_start(
                    wh[:, dt_ * 256:(dt_ + 1) * 256],
                    t["wv"][dt_ * P:(dt_ + 1) * P,
                            qe * 256:(qe + 1) * 256].bitcast(F32R))
            for st in range(ST):
                ps = psqkv.tile([P, 256], F32, tag="psv")
                for dt_ in range(DT):
                    nc.tensor.matmul(
                        ps[:],
                        hT[:, dt_ * T + st * P: dt_ * T + (st + 1) * P],
                        wh[:, dt_ * 256:(dt_ + 1) * 256],
                        start=(dt_ == 0), stop=(dt_ == DT - 1))
                nc.vector.tensor_copy(
                    v16[:, st * D + qe * 256: st * D + (qe + 1) * 256], ps[:])

    cm_hT.__exit__(None, None, None)   # hT no longer needed

    # ---------- Phase 4: LN1(xq) -> hTq; q^T = Wq^T hTq -------------------
    cm_qt = tc.tile_pool(name="qt", bufs=1)
    qt_pool = cm_qt.__enter__()
    qT = qt_pool.tile([P, DT * TQ], F32R)       # [e, t] 32KB/part
    with tc.tile_pool(name="hTq", bufs=1) as hTq_pool:
        hTq = hTq_pool.tile([P, DT * TQ], F32R)
        with tc.tile_pool(name="ln1q", bufs=2) as xpool, \
             tc.tile_pool(name="ln1qs", bufs=4) as statpool, \
             tc.tile_pool(name="ln1qp", bufs=4, space="PSUM") as pstr:
            _ln_transpose(nc, epsc, xq, TT, hTq[:], small["ln1g"][:],
                          small["ln1b"][:], idf, (xpool, statpool, pstr))
        with tc.tile_pool(name="wqh", bufs=2) as wpool, \
             tc.tile_pool(name="qp", bufs=4, space="PSUM") as psq:
            _proj_to_rows(nc, t["wq"], hTq[:], TQ, qT[:], TQ, psq, wpool)

    # ---------- Phase 5: attention ----------------------------------------
    cm_onorm = tc.tile_pool(name="onorm", bufs=1)
    onorm_pool = cm_onorm.__enter__()
    oT = onorm_pool.tile([P, DT * TQ], BF16)      # [e, t] unnormalized
    recips = onorm_pool.tile([P, H * TT], F32)    # col h*TT+i = 1/Z(h, t)

    with tc.tile_pool(name="psA", bufs=2, space="PSUM") as psA, \
         tc.tile_pool(name="psB", bufs=2, space="PSUM") as psB, \
         tc.tile_pool(name="psO", bufs=1, space="PSUM") as psO, \
         tc.tile_pool(name="expT", bufs=4) as expTp, \
         tc.tile_pool(name="ets", bufs=4) as etsp, \
         tc.tile_pool(name="att_s", bufs=4) as attsp:
        for p in range(H // 2):
            pair = (2 * p, 2 * p + 1)
            et = p   # both heads of the pair live in e-tile p
            # -- B side: scores [t, s], exp, Z, normalized attn out --
            # The two heads' K=64 matmuls are issued adjacently from
            # base_partition 0 / 64 so they run concurrently (row packing).
            for i in range(TT):
                ets2 = [etsp.tile([P, T], F32, tag="ets", name=f"ets_{p}_{i}_{j}")
                        for j in range(2)]
                sums42 = [attsp.tile([P, 4], F32, tag="s4", name=f"s4_{p}_{i}_{j}")
                          for j in range(2)]
                for c in range(4):
                    for hi, h in enumerate(pair):
                        lo = hi * HD
                        ps = psB.tile([P, 512], F32, tag="b")
                        nc.tensor.matmul(
                            ps[:],
                            qT[lo:lo + HD, et * TQ + i * P: et * TQ + (i + 1) * P],
                            kT[lo:lo + HD, et * T + c * 512: et * T + (c + 1) * 512],
                            start=True, stop=True)
                        nc.scalar.activation(
                            ets2[hi][:, c * 512:(c + 1) * 512], ps[:], AF.Exp,
                            bias=0.0, scale=SCALE,
                            accum_out=sums42[hi][:, c:c + 1])
                for hi, h in enumerate(pair):
                    z = attsp.tile([P, 1], F32, tag="z")
                    nc.vector.tensor_reduce(z[:], sums42[hi][:],
                                            axis=mybir.AxisListType.X, op=ALU.add)
                    rc = recips[:, h * TT + i: h * TT + i + 1]
                    nc.vector.reciprocal(rc, z[:])
                    nc.vector.tensor_scalar_mul(ets2[hi][:], ets2[hi][:], rc)
                    if "noattndma" not in ABLATE:
                        nc.sync.dma_start(attn[h, i * P:(i + 1) * P, :],
                                          ets2[hi][:])
            # -- A side: scores^T [s, t], exp, o accumulation --
            pso = psO.tile([P, TQ], F32, tag="o")
            nc.vector.memset(pso[:], 0.0)
            for k in range(ST):
                psa2 = [psA.tile([P, TQ], F32, tag="a", name=f"psa_{p}_{k}_{j}")
                        for j in range(2)]
                for ch in range(2):
                    for hi, h in enumerate(pair):
                        lo = hi * HD
                        nc.tensor.matmul(
                            psa2[hi][:, ch * 512:(ch + 1) * 512],
                            kT[lo:lo + HD, et * T + k * P: et * T + (k + 1) * P],
                            qT[lo:lo + HD, et * TQ + ch * 512: et * TQ + (ch + 1) * 512],
                            start=True, stop=True)
                for hi, h in enumerate(pair):
                    eT = expTp.tile([P, TQ], BF16, tag="eT")
                    nc.scalar.activation(eT[:], psa2[hi][:], AF.Exp,
                                         bias=0.0, scale=SCALE)
                    for ch in range(2):
                        nc.tensor.matmul(
                            pso[hi * HD:(hi + 1) * HD, ch * 512:(ch + 1) * 512],
                            v16[:, k * D + h * HD: k * D + (h + 1) * HD],
                            eT[:, ch * 512:(ch + 1) * 512],
                            start=False, stop=False,
                            tile_position=(0, hi * HD),
                            skip_group_check=True)
            nc.vector.tensor_copy(oT[:, p * TQ:(p + 1) * TQ], pso[:])
    # bounce oT / recips through DRAM so the pool stack can unwind
    cm_od = tc.tile_pool(name="od", bufs=1, space="DRAM")
    od_pool = cm_od.__enter__()
    oT_scr = od_pool.tile([P, DT * TQ], BF16)
    rc_scr = od_pool.tile([P, H * TT], F32)
    nc.sync.dma_start(oT_scr[:], oT[:])
    nc.sync.dma_start(rc_scr[:], recips[:])
    cm_onorm.__exit__(None, None, None)
    cm_qt.__exit__(None, None, None)
    cm_kv.__exit__(None, None, None)

    # re-establish pools for the tail: rT outermost, then x1, then onorm2
    cm_rT = tc.tile_pool(name="rT", bufs=1)
    rT_pool = cm_rT.__enter__()
    rT = rT_pool.tile([P, JT * TQ], BF16)   # [j, t] 64KB/part
    cm_x1 = tc.tile_pool(name="x1", bufs=1)
    x1_pool = cm_x1.__enter__()
    x1 = x1_pool.tile([P, TT * D], F32)
    cm_x1d = tc.tile_pool(name="x1d", bufs=1, space="DRAM")
    x1d_pool = cm_x1d.__enter__()
    x1_scr = x1d_pool.tile([TQ, D], F32)
    cm_onorm2 = tc.tile_pool(name="onorm2", bufs=1)
    onorm2_pool = cm_onorm2.__enter__()
    oT = onorm2_pool.tile([P, DT * TQ], BF16)
    recips = onorm2_pool.tile([P, H * TT], F32)
    nc.sync.dma_start(oT[:], oT_scr[:])
    nc.sync.dma_start(recips[:], rc_scr[:])

    # ---------- Phase 6: normalize o (double transpose) -------------------
    with tc.tile_pool(name="psT1", bufs=2, space="PSUM") as psT1, \
         tc.tile_pool(name="psT2", bufs=2, space="PSUM") as psT2, \
         tc.tile_pool(name="otmp", bufs=3) as otmp:
        for p in range(H // 2):
            for i in range(TT):
                sl = oT[:, p * TQ + i * P: p * TQ + (i + 1) * P]
                t1 = psT1.tile([P, P], BF16, tag="t1")
                nc.tensor.transpose(t1[:], sl, id16[:])
                m2 = otmp.tile([P, P], BF16, tag="m2")
                for hi in range(2):
                    h = 2 * p + hi
                    nc.vector.tensor_scalar_mul(
                        m2[:, hi * HD:(hi + 1) * HD],
                        t1[:, hi * HD:(hi + 1) * HD],
                        recips[:, h * TT + i: h * TT + i + 1])
                t2 = psT2.tile([P, P], BF16, tag="t2")
                nc.tensor.transpose(t2[:], m2[:], id16[:])
                nc.vector.tensor_copy(sl, t2[:])

    # ---------- Phase 7: x1 = xq + o Wp + bp ------------------------------
    with tc.tile_pool(name="wp_sb", bufs=1) as wpp, \
         tc.tile_pool(name="pswp", bufs=4, space="PSUM") as pswp, \
         tc.tile_pool(name="xqre", bufs=2) as xqre:
        wp_sb = wpp.tile([P, DT * D], BF16)
        for et in range(DT):
            nc.sync.dma_start(wp_sb[:, et * D:(et + 1) * D],
                              t["wp"][et * P:(et + 1) * P, :])
        for i in range(TT):
            xqt = xqre.tile([P, D], F32, tag="xq")
            nc.sync.dma_start(xqt[:], xq[i * P:(i + 1) * P, :])
            for c in range(2):
                ps = pswp.tile([P, 512], F32, tag="ps")
                for et in range(DT):
                    nc.tensor.matmul(
                        ps[:],
                        oT[:, et * TQ + i * P: et * TQ + (i + 1) * P],
                        wp_sb[:, et * D + c * 512: et * D + (c + 1) * 512],
                        start=(et == 0), stop=(et == DT - 1))
                sl = x1[:, i * D + c * 512: i * D + (c + 1) * 512]
                nc.vector.tensor_tensor(sl, ps[:], xqt[:, c * 512:(c + 1) * 512],
                                        op=ALU.add)
                nc.vector.tensor_tensor(sl, sl,
                                        bcast["bp"][:, c * 512:(c + 1) * 512],
                                        op=ALU.add)
            nc.sync.dma_start(x1_scr[i * P:(i + 1) * P, :],
                              x1[:, i * D:(i + 1) * D])
    cm_onorm2.__exit__(None, None, None)   # free oT / recips

    # ---------- Phase 8: LN2 + FFN first matmul + relu --------------------
    with tc.tile_pool(name="h2T", bufs=1) as h2_pool:
        h2T = h2_pool.tile([P, DT * TQ], F32R)
        with tc.tile_pool(name="ln2", bufs=3) as xpool, \
             tc.tile_pool(name="ln2s", bufs=4) as statpool, \
             tc.tile_pool(name="ln2p", bufs=4, space="PSUM") as pstr:
            _ln_transpose(nc, epsc, lambda i: x1[:, i * D:(i + 1) * D], TT,
                          h2T[:], small["ln2g"][:], small["ln2b"][:], idf,
                          (xpool, statpool, pstr))
        with tc.tile_pool(name="w1s", bufs=2) as w1p, \
             tc.tile_pool(name="psf1", bufs=2, space="PSUM") as psf1:
            for q8 in range(8):
                w1t = w1p.tile([P, DT * 512], F32R, tag="w1")
                for dt_ in range(DT):
                    nc.sync.dma_start(
                        w1t[:, dt_ * 512:(dt_ + 1) * 512],
                        t["w1"][dt_ * P:(dt_ + 1) * P,
                                q8 * 512:(q8 + 1) * 512].bitcast(F32R))
                for jl in range(4):
                    jt = q8 * 4 + jl
                    ps = psf1.tile([P, TQ], F32, tag="ps")
                    for ch in range(2):
                        for dt_ in range(DT):
                            nc.tensor.matmul(
                                ps[:, ch * 512:(ch + 1) * 512],
                                w1t[:, dt_ * 512 + jl * P: dt_ * 512 + (jl + 1) * P],
                                h2T[:, dt_ * TQ + ch * 512: dt_ * TQ + (ch + 1) * 512],
                                start=(dt_ == 0), stop=(dt_ == DT - 1))
                    nc.vector.tensor_scalar(
                        rT[:, jt * TQ:(jt + 1) * TQ], ps[:],
                        small["b1"][:, jt:jt + 1], 0.0,
                        op0=ALU.add, op1=ALU.max)
    cm_x1.__exit__(None, None, None)      # x1 now only in DRAM scratch

    # ---------- Phase 9: out = x1 + rT^T W2 + b2 --------------------------
    with tc.tile_pool(name="w2sb", bufs=1) as w2p, \
         tc.tile_pool(name="psf2", bufs=2, space="PSUM") as psf2, \
         tc.tile_pool(name="x1re", bufs=2) as x1re, \
         tc.tile_pool(name="yout", bufs=2) as youtp:
        w2sb = w2p.tile([P, JT * D], BF16)
        for jt in range(JT):
            nc.sync.dma_start(w2sb[:, jt * D:(jt + 1) * D],
                              t["w2"][jt * P:(jt + 1) * P, :])
        for i in range(TT):
            x1t = x1re.tile([P, D], F32, tag="x1")
            nc.sync.dma_start(x1t[:], x1_scr[i * P:(i + 1) * P, :])
            ps = psf2.tile([P, D], F32, tag="ps")
            for ch in range(2):
                for jt in range(JT):
                    nc.tensor.matmul(
                        ps[:, ch * 512:(ch + 1) * 512],
                        rT[:, jt * TQ + i * P: jt * TQ + (i + 1) * P],
                        w2sb[:, jt * D + ch * 512: jt * D + (ch + 1) * 512],
                        start=(jt == 0), stop=(jt == JT - 1))
            yt = youtp.tile([P, D], F32, tag="y")
            nc.vector.tensor_tensor(yt[:], ps[:], x1t[:], op=ALU.add)
            nc.vector.tensor_tensor(yt[:], yt[:], bcast["b2"][:], op=ALU.add)
            nc.sync.dma_start(y[i * P:(i + 1) * P, :], yt[:])

    cm_rT.__exit__(None, None, None)
    cm_x1d.__exit__(None, None, None)
    cm_od.__exit__(None, None, None)
    cm_const.__exit__(None, None, None)


_NC = None


def _get_nc():
    global _NC
    if _NC is None:
        _NC = build_nc()
    return _NC


def make_in_maps(inputs):
    f32 = np.float32
    bf16 = ml_dtypes.bfloat16
    x = np.asarray(inputs["x"], f32)
    shared = {
        "ln1g": np.asarray(inputs["ln1_g"], f32),
        "ln1b": np.asarray(inputs["ln1_b"], f32),
        "ln2g": np.asarray(inputs["ln2_g"], f32),
        "ln2b": np.asarray(inputs["ln2_b"], f32),
        "wq": np.ascontiguousarray(
            np.asarray(inputs["Wq"], f32).transpose(1, 0, 2).reshape(D, D)),
        "wk": np.ascontiguousarray(
            np.asarray(inputs["Wk"], f32).transpose(1, 0, 2).reshape(D, D)),
        "wv": np.ascontiguousarray(
            np.asarray(inputs["Wv"], f32).transpose(1, 0, 2).reshape(D, D)),
        "wp": np.asarray(inputs["Wp"], f32).astype(bf16),
        "bp": np.asarray(inputs["bp"], f32),
        "w1": np.asarray(inputs["W1"], f32),
        "b1": np.asarray(inputs["b1"], f32),
        "w2": np.asarray(inputs["W2"], f32).astype(bf16),
        "b2": np.asarray(inputs["b2"], f32),
    }
    in_maps = []
    for c in range(8):
        b, half = c // 2, c % 2
        m = dict(shared)
        m["xf"] = np.ascontiguousarray(x[b])
        m["xq"] = np.ascontiguousarray(x[b, half * TQ:(half + 1) * TQ])
        in_maps.append(m)
    return in_maps


def assemble(results):
    y_full = np.empty((B, T, D), np.float32)
    attn_full = np.empty((H, B, T, T), np.float32)
    for c, r in enumerate(results):
        b, half = c // 2, c % 2
        y_full[b, half * TQ:(half + 1) * TQ] = r["y"]
        attn_full[:, b, half * TQ:(half + 1) * TQ, :] = r["attn"]
    return y_full, attn_full


def run(inputs, **kwargs):
    nc = _get_nc()
    res = run_bass_kernel_spmd(nc, make_in_maps(inputs),
                               core_ids=list(range(8)), **kwargs)
    return res


def kernel(**inputs):
    res = run(inputs)
    return assemble(res.results)


# revision 23
# speedup vs baseline: 1.8922x; 1.8922x over previous
"""Trainium2 Bass kernel for nn_Block_45518063403599 (dense transformer block).

Full inputs -> full outputs. Sharding: 8 cores = (batch b in 0..3) x (query
half in 0..1). Each core computes the block for its 1024 query tokens of its
batch (k/v over all 2048 tokens of that batch are recomputed per core pair --
zero cross-core communication, single SPMD launch).

Reference (eval mode):
    h  = LN1(x);  q,k,v = h @ Wq/Wk/Wv (per-head)
    attn = softmax(q k^T / 8);  o = attn @ v
    x1 = x + concat_heads(o) @ Wp + bp
    ff = relu(LN2(x1) @ W1 + b1) @ W2 + b2
    out = (x1 + ff, attn stacked [H,B,T,T])
"""
import numpy as np
import ml_dtypes

import concourse.bass as bass
import concourse.tile as tile
import concourse.mybir as mybir
from concourse import bacc
from concourse.bass_utils import run_bass_kernel_spmd
from concourse.masks import make_identity

F32 = mybir.dt.float32
F32R = mybir.dt.float32r
BF16 = mybir.dt.bfloat16
AF = mybir.ActivationFunctionType
ALU = mybir.AluOpType

B, T, D, H = 4, 2048, 1024, 16
HD = D // H            # 64
TQ = T // 2            # 1024 query tokens per core
J = 4 * D              # 4096
P = 128
DT = D // P            # 8 d-tiles
TT = TQ // P           # 8 query-token tiles
ST = T // P            # 16 kv-token tiles
JT = J // P            # 32 ffn-hidden tiles
LN_EPS = 1e-5
SCALE = float(HD) ** -0.5   # 1/8


def _ln_transpose(nc, eps, xsrc, ntiles, dst, g, b, idf, pools):
    """LayerNorm rows of xsrc and write the transposed result into dst
    ([128, DT*ntiles*128] f32r), applying per-d affine (g, b) post-transpose.
    xsrc: either a DRAM AP [ntiles*128, D] or a fn(i)->SBUF AP [128, D]."""
    xpool, statpool, pstr = pools
    ncols = ntiles * P
    for i in range(ntiles):
        if callable(xsrc):
            xt = xsrc(i)
        else:
            xtile = xpool.tile([P, D], F32, tag="ln_x")
            nc.sync.dma_start(xtile[:], xsrc[i * P:(i + 1) * P, :])
            xt = xtile[:]
        s1 = statpool.tile([P, 1], F32, tag="ln_s1")
        nc.vector.tensor_reduce(s1[:], xt, axis=mybir.AxisListType.X, op=ALU.add)
        negmu = statpool.tile([P, 1], F32, tag="ln_negmu")
        nc.vector.tensor_scalar_mul(negmu[:], s1[:], -1.0 / D)
        sq = xpool.tile([P, D], F32, tag="ln_sq")
        ssq = statpool.tile([P, 1], F32, tag="ln_ssq")
        nc.scalar.activation(sq[:], xt, AF.Square, bias=negmu[:], scale=1.0,
                             accum_out=ssq[:])
        std = statpool.tile([P, 1], F32, tag="ln_std")
        nc.scalar.activation(std[:], ssq[:], AF.Sqrt, bias=eps[:], scale=1.0 / D)
        rstd = statpool.tile([P, 1], F32, tag="ln_rstd")
        nc.vector.reciprocal(rstd[:], std[:])
        cn = xpool.tile([P, D], F32, tag="ln_cn")
        nc.vector.tensor_scalar(cn[:], xt, negmu[:], rstd[:],
                                op0=ALU.add, op1=ALU.mult)
        for k in range(DT):
            pt = pstr.tile([P, P], F32, tag="ln_ps")
            nc.tensor.transpose(pt[:], cn[:, k * P:(k + 1) * P], idf[:])
            nc.vector.tensor_scalar(
                dst[:, k * ncols + i * P: k * ncols + (i + 1) * P], pt[:],
                g[:, k:k + 1], b[:, k:k + 1], op0=ALU.mult, op1=ALU.add)


def _proj_to_rows(nc, w_dram, src, src_cols, dst, dst_cols, psp, wpool):
    """dst[e, n] = sum_d w[d, e] * src[d, n] for e in 0..1023 (f32r matmuls).

    src: SBUF [128, DT*src_cols] f32r (col-block = d-tile).
    dst: SBUF [128, DT*dst_cols] (col-block = e-tile); dst_cols==src_cols.
    Streams w in 256-wide e-quarters."""
    nch = src_cols // 512
    for qe in range(4):
        wh = wpool.tile([P, DT * 256], F32R, tag="w")
        for dt_ in range(DT):
            nc.sync.dma_start(
                wh[:, dt_ * 256:(dt_ + 1) * 256],
                w_dram[dt_ * P:(dt_ + 1) * P,
                       qe * 256:(qe + 1) * 256].bitcast(F32R))
        for el in range(2):
            et = qe * 2 + el
            for ch in range(nch):
                ps = psp.tile([P, 512], F32, tag="ps")
                for dt_ in range(DT):
                    nc.tensor.matmul(
                        ps[:],
                        wh[:, dt_ * 256 + el * P: dt_ * 256 + (el + 1) * P],
                        src[:, dt_ * src_cols + ch * 512: dt_ * src_cols + (ch + 1) * 512],
                        start=(dt_ == 0), stop=(dt_ == DT - 1))
                nc.vector.tensor_copy(
                    dst[:, et * dst_cols + ch * 512: et * dst_cols + (ch + 1) * 512],
                    ps[:])


def build_nc():
    nc = bacc.Bacc("TRN2", target_bir_lowering=False, debug=False)

    t = {}
    t["xf"] = nc.dram_tensor("xf", [T, D], F32, kind="ExternalInput").ap()
    t["xq"] = nc.dram_tensor("xq", [TQ, D], F32, kind="ExternalInput").ap()
    for n in ["ln1g", "ln1b", "ln2g", "ln2b", "bp", "b2"]:
        t[n] = nc.dram_tensor(n, [D], F32, kind="ExternalInput").ap()
    t["b1"] = nc.dram_tensor("b1", [J], F32, kind="ExternalInput").ap()
    for n in ["wq", "wk", "wv"]:
        t[n] = nc.dram_tensor(n, [D, D], F32, kind="ExternalInput").ap()
    t["wp"] = nc.dram_tensor("wp", [D, D], BF16, kind="ExternalInput").ap()
    t["w1"] = nc.dram_tensor("w1", [D, J], F32, kind="ExternalInput").ap()
    t["w2"] = nc.dram_tensor("w2", [J, D], BF16, kind="ExternalInput").ap()
    t["y"] = nc.dram_tensor("y", [TQ, D], F32, kind="ExternalOutput").ap()
    t["attn"] = nc.dram_tensor("attn", [H, TQ, T], F32, kind="ExternalOutput").ap()

    with tile.TileContext(nc) as tc:
        _emit(nc, tc, t)
    nc.compile()
    return nc


def _emit(nc, tc, t):
    xf, xq, y, attn = t["xf"], t["xq"], t["y"], t["attn"]

    cm_const = tc.tile_pool(name="const", bufs=1)
    const = cm_const.__enter__()
    idf = const.tile([P, P], F32)
    make_identity(nc, idf[:])
    id16 = const.tile([P, P], BF16)
    make_identity(nc, id16[:])
    epsc = const.tile([P, 1], F32)
    nc.gpsimd.memset(epsc[:], LN_EPS)
    small = {}
    for name, n in [("ln1g", D), ("ln1b", D), ("ln2g", D), ("ln2b", D),
                    ("b1", J)]:
        s = const.tile([P, n // P], F32, tag=name)
        nc.sync.dma_start(s[:], t[name].rearrange("(k p) -> p k", p=P))
        small[name] = s
    bcast = {}
    for name in ["bp", "b2"]:
        row = const.tile([1, D], F32, tag=f"{name}_row")
        nc.sync.dma_start(row[:], t[name][None, :])
        full = const.tile([P, D], F32, tag=f"{name}_b")
        nc.gpsimd.partition_broadcast(full[:], row[:])
        bcast[name] = full

    # ---------- Phase 1: LN1(xf) -> hT [d, s] f32r (64KB/part) ------------
    cm_kv = tc.tile_pool(name="kv", bufs=1)
    kv_pool = cm_kv.__enter__()
    kT = kv_pool.tile([P, DT * T], F32R)        # [e, s] 64KB/part
    v16 = kv_pool.tile([P, ST * D], BF16)       # [s, e] 32KB/part

    cm_hT = tc.tile_pool(name="hT", bufs=1)
    hT_pool = cm_hT.__enter__()
    hT = hT_pool.tile([P, DT * T], F32R)
    with tc.tile_pool(name="ln1", bufs=2) as xpool, \
         tc.tile_pool(name="ln1s", bufs=4) as statpool, \
         tc.tile_pool(name="ln1p", bufs=4, space="PSUM") as pstr:
        _ln_transpose(nc, epsc, xf, ST, hT[:], small["ln1g"][:],
                      small["ln1b"][:], idf, (xpool, statpool, pstr))

    # ---------- Phase 2: k^T = Wk^T h^T -----------------------------------
    with tc.tile_pool(name="wkh", bufs=2) as wpool, \
         tc.tile_pool(name="qkvp", bufs=4, space="PSUM") as psqkv:
        _proj_to_rows(nc, t["wk"], hT[:], T, kT[:], T, psqkv, wpool)
        # ---------- Phase 3: v = h Wv (layout [s, e]) ---------------------
        for qe in range(4):
            wh = wpool.tile([P, DT * 256], F32R, tag="w")
            for dt_ in range(DT):
                nc.sync.dma_start(
                    wh[:, dt_ * 256:(dt_ + 1) * 256],
                    t["wv"][dt_ * P:(dt_ + 1) * P,
                            qe * 256:(qe + 1) * 256].bitcast(F32R))
            for st in range(ST):
                ps = psqkv.tile([P, 256], F32, tag="psv")
                for dt_ in range(DT):
                    nc.tensor.matmul(
                        ps[:],
                        hT[:, dt_ * T + st * P: dt_ * T + (st + 1) * P],
                        wh[:, dt_ * 256:(dt_ + 1) * 256],
                        start=(dt_ == 0), stop=(dt_ == DT - 1))
                nc.vector.tensor_copy(
                    v16[:, st * D + qe * 256: st * D + (qe + 1) * 256], ps[:])

    cm_hT.__exit__(None, None, None)   # hT no longer needed

    # ---------- Phase 4: LN1(xq) -> hTq; q^T = Wq^T hTq -------------------
    cm_qt = tc.tile_pool(name="qt", bufs=1)
    qt_pool = cm_qt.__enter__()
    qT = qt_pool.tile([P, DT * TQ], F32R)       # [e, t] 32KB/part
    with tc.tile_pool(name="hTq", bufs=1) as hTq_pool:
        hTq = hTq_pool.tile([P, DT * TQ], F32R)
        with tc.tile_pool(name="ln1q", bufs=2) as xpool, \
             tc.tile_pool(name="ln1qs", bufs=4) as statpool, \
             tc.tile_pool(name="ln1qp", bufs=4, space="PSUM") as pstr:
            _ln_transpose(nc, epsc, xq, TT, hTq[:], small["ln1g"][:],
                          small["ln1b"][:], idf, (xpool, statpool, pstr))
        with tc.tile_pool(name="wqh", bufs=2) as wpool, \
             tc.tile_pool(name="qp", bufs=4, space="PSUM") as psq:
            _proj_to_rows(nc, t["wq"], hTq[:], TQ, qT[:], TQ, psq, wpool)

    # ---------- Phase 5: attention ----------------------------------------
    cm_onorm = tc.tile_pool(name="onorm", bufs=1)
    onorm_pool = cm_onorm.__enter__()
    oT = onorm_pool.tile([P, DT * TQ], BF16)      # [e, t] unnormalized
    recips = onorm_pool.tile([P, H * TT], F32)    # col h*TT+i = 1/Z(h, t)

    with tc.tile_pool(name="psA", bufs=2, space="PSUM") as psA, \
         tc.tile_pool(name="psB", bufs=2, space="PSUM") as psB, \
         tc.tile_pool(name="psO", bufs=1, space="PSUM") as psO, \
         tc.tile_pool(name="expT", bufs=4) as expTp, \
         tc.tile_pool(name="ets", bufs=4) as etsp, \
         tc.tile_pool(name="att_s", bufs=4) as attsp:
        for p in range(H // 2):
            pair = (2 * p, 2 * p + 1)
            et = p   # both heads of the pair live in e-tile p
            # -- B side: scores [t, s], exp, Z, normalized attn out --
            for hi, h in enumerate(pair):
                lo = hi * HD
                for i in range(TT):
                    ets = etsp.tile([P, T], F32, tag="ets")
                    sums4 = attsp.tile([P, 4], F32, tag="s4")
                    for c in range(4):
                        ps = psB.tile([P, 512], F32, tag="b")
                        nc.tensor.matmul(
                            ps[:],
                            qT[lo:lo + HD, et * TQ + i * P: et * TQ + (i + 1) * P],
                            kT[lo:lo + HD, et * T + c * 512: et * T + (c + 1) * 512],
                            start=True, stop=True)
                        nc.scalar.activation(
                            ets[:, c * 512:(c + 1) * 512], ps[:], AF.Exp,
                            bias=0.0, scale=SCALE, accum_out=sums4[:, c:c + 1])
                    z = attsp.tile([P, 1], F32, tag="z")
                    nc.vector.tensor_reduce(z[:], sums4[:],
                                            axis=mybir.AxisListType.X, op=ALU.add)
                    rc = recips[:, h * TT + i: h * TT + i + 1]
                    nc.vector.reciprocal(rc, z[:])
                    nc.vector.tensor_scalar_mul(ets[:], ets[:], rc)
                    nc.sync.dma_start(attn[h, i * P:(i + 1) * P, :], ets[:])
            # -- A side: scores^T [s, t], exp, o accumulation --
            pso = psO.tile([P, TQ], F32, tag="o")
            nc.vector.memset(pso[:], 0.0)
            for k in range(ST):
                for hi, h in enumerate(pair):
                    lo = hi * HD
                    psa = psA.tile([P, TQ], F32, tag="a")
                    for ch in range(2):
                        nc.tensor.matmul(
                            psa[:, ch * 512:(ch + 1) * 512],
                            kT[lo:lo + HD, et * T + k * P: et * T + (k + 1) * P],
                            qT[lo:lo + HD, et * TQ + ch * 512: et * TQ + (ch + 1) * 512],
                            start=True, stop=True)
                    eT = expTp.tile([P, TQ], BF16, tag="eT")
                    nc.scalar.activation(eT[:], psa[:], AF.Exp,
                                         bias=0.0, scale=SCALE)
                    for ch in range(2):
                        nc.tensor.matmul(
                            pso[hi * HD:(hi + 1) * HD, ch * 512:(ch + 1) * 512],
                            v16[:, k * D + h * HD: k * D + (h + 1) * HD],
                            eT[:, ch * 512:(ch + 1) * 512],
                            start=False, stop=False,
                            tile_position=(0, hi * HD),
                            skip_group_check=True)
            nc.vector.tensor_copy(oT[:, p * TQ:(p + 1) * TQ], pso[:])
    # bounce oT / recips through DRAM so the pool stack can unwind
    cm_od = tc.tile_pool(name="od", bufs=1, space="DRAM")
    od_pool = cm_od.__enter__()
    oT_scr = od_pool.tile([P, DT * TQ], BF16)
    rc_scr = od_pool.tile([P, H * TT], F32)
    nc.sync.dma_start(oT_scr[:], oT[:])
    nc.sync.dma_start(rc_scr[:], recips[:])
    cm_onorm.__exit__(None, None, None)
    cm_qt.__exit__(None, None, None)
    cm_kv.__exit__(None, None, None)

    # re-establish pools for the tail: rT outermost, then x1, then onorm2
    cm_rT = tc.tile_pool(name="rT", bufs=1)
    rT_pool = cm_rT.__enter__()
    rT = rT_pool.tile([P, JT * TQ], BF16)   # [j, t] 64KB/part
    cm_x1 = tc.tile_pool(name="x1", bufs=1)
    x1_pool = cm_x1.__enter__()
    x1 = x1_pool.tile([P, TT * D], F32)
    cm_x1d = tc.tile_pool(name="x1d", bufs=1, space="DRAM")
    x1d_pool = cm_x1d.__enter__()
    x1_scr = x1d_pool.tile([TQ, D], F32)
    cm_onorm2 = tc.tile_pool(name="onorm2", bufs=1)
    onorm2_pool = cm_onorm2.__enter__()
    oT = onorm2_pool.tile([P, DT * TQ], BF16)
    recips = onorm2_pool.tile([P, H * TT], F32)
    nc.sync.dma_start(oT[:], oT_scr[:])
    nc.sync.dma_start(recips[:], rc_scr[:])

    # ---------- Phase 6: normalize o (double transpose) -------------------
    with tc.tile_pool(name="psT1", bufs=2, space="PSUM") as psT1, \
         tc.tile_pool(name="psT2", bufs=2, space="PSUM") as psT2, \
         tc.tile_pool(name="otmp", bufs=3) as otmp:
        for p in range(H // 2):
            for i in range(TT):
                sl = oT[:, p * TQ + i * P: p * TQ + (i + 1) * P]
                t1 = psT1.tile([P, P], BF16, tag="t1")
                nc.tensor.transpose(t1[:], sl, id16[:])
                m2 = otmp.tile([P, P], BF16, tag="m2")
                for hi in range(2):
                    h = 2 * p + hi
                    nc.vector.tensor_scalar_mul(
                        m2[:, hi * HD:(hi + 1) * HD],
                        t1[:, hi * HD:(hi + 1) * HD],
                        recips[:, h * TT + i: h * TT + i + 1])
                t2 = psT2.tile([P, P], BF16, tag="t2")
                nc.tensor.transpose(t2[:], m2[:], id16[:])
                nc.vector.tensor_copy(sl, t2[:])

    # ---------- Phase 7: x1 = xq + o Wp + bp ------------------------------
    with tc.tile_pool(name="wp_sb", bufs=1) as wpp, \
         tc.tile_pool(name="pswp", bufs=4, space="PSUM") as pswp, \
         tc.tile_pool(name="xqre", bufs=2) as xqre:
        wp_sb = wpp.tile([P, DT * D], BF16)
        for et in range(DT):
            nc.sync.dma_start(wp_sb[:, et * D:(et + 1) * D],
                              t["wp"][et * P:(et + 1) * P, :])
        for i in range(TT):
            xqt = xqre.tile([P, D], F32, tag="xq")
            nc.sync.dma_start(xqt[:], xq[i * P:(i + 1) * P, :])
            for c in range(2):
                ps = pswp.tile([P, 512], F32, tag="ps")
                for et in range(DT):
                    nc.tensor.matmul(
                        ps[:],
                        oT[:, et * TQ + i * P: et * TQ + (i + 1) * P],
                        wp_sb[:, et * D + c * 512: et * D + (c + 1) * 512],
                        start=(et == 0), stop=(et == DT - 1))
                sl = x1[:, i * D + c * 512: i * D + (c + 1) * 512]
                nc.vector.tensor_tensor(sl, ps[:], xqt[:, c * 512:(c + 1) * 512],
                                        op=ALU.add)
                nc.vector.tensor_tensor(sl, sl,
                                        bcast["bp"][:, c * 512:(c + 1) * 512],
                                        op=ALU.add)
            nc.sync.dma_start(x1_scr[i * P:(i + 1) * P, :],
                              x1[:, i * D:(i + 1) * D])
    cm_onorm2.__exit__(None, None, None)   # free oT / recips

    # ---------- Phase 8: LN2 + FFN first matmul + relu --------------------
    with tc.tile_pool(name="h2T", bufs=1) as h2_pool:
        h2T = h2_pool.tile([P, DT * TQ], F32R)
        with tc.tile_pool(name="ln2", bufs=3) as xpool, \
             tc.tile_pool(name="ln2s", bufs=4) as statpool, \
             tc.tile_pool(name="ln2p", bufs=4, space="PSUM") as pstr:
            _ln_transpose(nc, epsc, lambda i: x1[:, i * D:(i + 1) * D], TT,
                          h2T[:], small["ln2g"][:], small["ln2b"][:], idf,
                          (xpool, statpool, pstr))
        with tc.tile_pool(name="w1s", bufs=2) as w1p, \
             tc.tile_pool(name="psf1", bufs=2, space="PSUM") as psf1:
            for q8 in range(8):
                w1t = w1p.tile([P, DT * 512], F32R, tag="w1")
                for dt_ in range(DT):
                    nc.sync.dma_start(
                        w1t[:, dt_ * 512:(dt_ + 1) * 512],
                        t["w1"][dt_ * P:(dt_ + 1) * P,
                                q8 * 512:(q8 + 1) * 512].bitcast(F32R))
                for jl in range(4):
                    jt = q8 * 4 + jl
                    ps = psf1.tile([P, TQ], F32, tag="ps")
                    for ch in range(2):
                        for dt_ in range(DT):
                            nc.tensor.matmul(
                                ps[:, ch * 512:(ch + 1) * 512],
                                w1t[:, dt_ * 512 + jl * P: dt_ * 512 + (jl + 1) * P],
                                h2T[:, dt_ * TQ + ch * 512: dt_ * TQ + (ch + 1) * 512],
                                start=(dt_ == 0), stop=(dt_ == DT - 1))
                    nc.vector.tensor_scalar(
                        rT[:, jt * TQ:(jt + 1) * TQ], ps[:],
                        small["b1"][:, jt:jt + 1], 0.0,
                        op0=ALU.add, op1=ALU.max)
    cm_x1.__exit__(None, None, None)      # x1 now only in DRAM scratch

    # ---------- Phase 9: out = x1 + rT^T W2 + b2 --------------------------
    with tc.tile_pool(name="w2sb", bufs=1) as w2p, \
         tc.tile_pool(name="psf2", bufs=2, space="PSUM") as psf2, \
         tc.tile_pool(name="x1re", bufs=2) as x1re, \
         tc.tile_pool(name="yout", bufs=2) as youtp:
        w2sb = w2p.tile([P, JT * D], BF16)
        for jt in range(JT):
            nc.sync.dma_start(w2sb[:, jt * D:(jt + 1) * D],
                              t["w2"][jt * P:(jt + 1) * P, :])
        for i in range(TT):
            x1t = x1re.tile([P, D], F32, tag="x1")
            nc.sync.dma_start(x1t[:], x1_scr[i * P:(i + 1) * P, :])
            ps = psf2.tile([P, D], F32, tag="ps")
            for ch in range(2):
                for jt in range(JT):
                    nc.tensor.matmul(
                        ps[:, ch * 512:(ch + 1) * 512],
                        rT[:, jt * TQ + i * P: jt * TQ + (i + 1) * P],
                        w2sb[:, jt * D + ch * 512: jt * D + (ch + 1) * 512],
                        start=(jt == 0), stop=(jt == JT - 1))
            yt = youtp.tile([P, D], F32, tag="y")
            nc.vector.tensor_tensor(yt[:], ps[:], x1t[:], op=ALU.add)
            nc.vector.tensor_tensor(yt[:], yt[:], bcast["b2"][:], op=ALU.add)
            nc.sync.dma_start(y[i * P:(i + 1) * P, :], yt[:])

    cm_rT.__exit__(None, None, None)
    cm_x1d.__exit__(None, None, None)
    cm_od.__exit__(None, None, None)
    cm_const.__exit__(None, None, None)


_NC = None


def _get_nc():
    global _NC
    if _NC is None:
        _NC = build_nc()
    return _NC


def make_in_maps(inputs):
    f32 = np.float32
    bf16 = ml_dtypes.bfloat16
    x = np.asarray(inputs["x"], f32)
    shared = {
        "ln1g": np.asarray(inputs["ln1_g"], f32),
        "ln1b": np.asarray(inputs["ln1_b"], f32),
        "ln2g": np.asarray(inputs["ln2_g"], f32),
        "ln2b": np.asarray(inputs["ln2_b"], f32),
        "wq": np.ascontiguousarray(
            np.asarray(inputs["Wq"], f32).transpose(1, 0, 2).reshape(D, D)),
        "wk": np.ascontiguousarray(
            np.asarray(inputs["Wk"], f32).transpose(1, 0, 2).reshape(D, D)),
        "wv": np.ascontiguousarray(
            np.asarray(inputs["Wv"], f32).transpose(1, 0, 2).reshape(D, D)),
        "wp": np.asarray(inputs["Wp"], f32).astype(bf16),
        "bp": np.asarray(inputs["bp"], f32),
        "w1": np.asarray(inputs["W1"], f32),
        "b1": np.asarray(inputs["b1"], f32),
        "w2": np.asarray(inputs["W2"], f32).astype(bf16),
        "b2": np.asarray(inputs["b2"], f32),
    }
    in_maps = []
    for c in range(8):
        b, half = c // 2, c % 2
        m = dict(shared)
        m["xf"] = np.ascontiguousarray(x[b])
        m["xq"] = np.ascontiguousarray(x[b, half * TQ:(half + 1) * TQ])
        in_maps.append(m)
    return in_maps


def assemble(results):
    y_full = np.empty((B, T, D), np.float32)
    attn_full = np.empty((H, B, T, T), np.float32)
    for c, r in enumerate(results):
        b, half = c // 2, c % 2
        y_full[b, half * TQ:(half + 1) * TQ] = r["y"]
        attn_full[:, b, half * TQ:(half + 1) * TQ, :] = r["attn"]
    return y_full, attn_full


def run(inputs, **kwargs):
    nc = _get_nc()
    res = run_bass_kernel_spmd(nc, make_in_maps(inputs),
                               core_ids=list(range(8)), **kwargs)
    return res


def kernel(**inputs):
    res = run(inputs)
    return assemble(res.results)
